# revision 53
# baseline (speedup 1.0000x reference)
"""Criss-cross attention block kernel for Trainium2 (Bass/Tile), 8-core data parallel.

Problem (hardcoded shapes): x [8, 96, 96, 512] fp32.
  q = x@Wq+bq, k = x@Wk+bk (c=64), v = x@Wv+bv (C=512)
  per pixel (h,w): softmax over [scores vs column (diag masked), scores vs row],
  y = gamma * (att_v + att_h) + x.

Sharding: pure data parallel, one batch image per NeuronCore (B=8 = n_cores).

Per-core algorithm:
  Phase 1: stream x in 256-pixel tiles; PE-transpose to get xT (C on partitions);
           project qT,kT (fp16, SBUF resident [64, 9216]) and v (bf16 -> HBM scratch).
  Phase 2 (per column w): S^T = K_w @ Q_w^T via matmul(lhsT=kT_w, rhs=qT_w);
           E = exp(S^T) (no max subtraction: |s| <= ~60, fp32/bf16 exponent safe);
           zero diagonal (h==g mask); yv_w = E^T.T @ V_w -> unnormalized, bf16 into
           SBUF-resident yv image; Zv_w = E^T.T @ ones -> Zv image.
  Phase 3 (per row r): same for rows (no mask); accumulate yv row (gathered from
           SBUF image via SBUF->SBUF DMA repartition) into the yh PSUM via an
           identity matmul; y = (psum * gamma/(Zv+Zh)) + x (one fused DVE op).
"""

import os

import numpy as np
import ml_dtypes

import concourse.bass as bass
import concourse.mybir as mybir
import concourse.tile as tile
from concourse import bacc
from concourse.bass import ts, ds
from concourse.masks import make_identity
from concourse.bass_utils import run_bass_kernel_spmd

F32 = mybir.dt.float32
F32R = mybir.dt.float32r
BF16 = mybir.dt.bfloat16
FP16 = mybir.dt.float16
F8 = mybir.dt.float8e4

H = 96
W = 96
C = 512
CQK = 64
NPIX = H * W  # 9216
N_CORES = 8

# phase-1 pixel tiling: PT pixels per tile, PS = PT//128 subchunks
PT = 512
PS = PT // 128
NT = NPIX // PT  # 36
KC = C // 128  # 4 contraction chunks
# phase-2/3 blocking (columns/rows per block)
WB = 8
RB = 4


def _r(ap):
    return ap.bitcast(F32R)


def build_nc() -> bass.Bass:
    nc = bacc.Bacc(
        "TRN2", target_bir_lowering=False, debug=False, num_devices=N_CORES
    )

    x = nc.dram_tensor("x", [NPIX, C], F32, kind="ExternalInput")[:]
    Wq = nc.dram_tensor("Wq", [C, CQK], F32, kind="ExternalInput")[:]
    bq = nc.dram_tensor("bq", [CQK], F32, kind="ExternalInput")[:]
    Wk = nc.dram_tensor("Wk", [C, CQK], F32, kind="ExternalInput")[:]
    bk = nc.dram_tensor("bk", [CQK], F32, kind="ExternalInput")[:]
    Wv = nc.dram_tensor("Wv", [C, C], F32, kind="ExternalInput")[:]
    bv = nc.dram_tensor("bv", [C], F32, kind="ExternalInput")[:]
    gamma = nc.dram_tensor("gamma", [1, 1], F32, kind="ExternalInput")[:]
    y = nc.dram_tensor("y", [NPIX, C], F32, kind="ExternalOutput")[:]

    with tile.TileContext(nc) as tc:
        _body(nc, tc, x, Wq, bq, Wk, bk, Wv, bv, gamma, y)
    nc.compile()
    return nc


def _body(nc, tc, x, Wq, bq, Wk, bk, Wv, bv, gamma, y):
    mult = mybir.AluOpType.mult
    add = mybir.AluOpType.add

    with (
        tc.tile_pool(name="singles", bufs=1) as singles,
        tc.tile_pool(name="dram", bufs=1, space="DRAM") as dram,
    ):
        # ---- constants / weights resident in SBUF ----
        identity128 = singles.tile([128, 128], F32)
        make_identity(nc, identity128)
        identity_h = singles.tile([128, 128], FP16)
        make_identity(nc, identity_h)
        id96b = singles.tile([96, 96], BF16)
        make_identity(nc, id96b)
        # anti-diagonal mask: 1 everywhere, 0 on diag
        antidiag = singles.tile([96, 96], BF16)
        nc.vector.memset(antidiag, 1.0)
        nc.gpsimd.affine_select(
            out=antidiag,
            in_=antidiag,
            compare_op=mybir.AluOpType.not_equal,
            fill=0.0,
            base=0,
            pattern=[[-1, 96]],
            channel_multiplier=1,
        )
        ones96 = singles.tile([96, 1], BF16)
        nc.vector.memset(ones96, 1.0)

        # weights: DMA fp32 staging, round to fp32r tiles (separate memlocs;
        # the fp32r verifier requires fp32r matmul operands to be produced by
        # a rounding instruction on that memory location). All tiles live in
        # the never-released `singles` pool so no SBUF address reuse creates
        # multi-sem WAR waits on later DMAs (HW DMA descriptors have a small
        # sync-wait slot limit).
        # fused q|k weight: wqk[p, cc, 0:64]=Wq, [p, cc, 64:128]=Wk
        wqk_sb = singles.tile([128, KC, 2 * CQK], FP16)
        wv_sb = singles.tile([128, KC, C], FP16)
        # ones column for the bias matmul (K=1): out += ones[1,M] (x) bvT[1,C]
        ones1 = singles.tile([1, 128], FP16)
        nc.vector.memset(ones1, 1.0)
        bv_row = singles.tile([1, C], FP16)
        with tc.tile_pool(name="wstage", bufs=1) as wstage:
            wq_f32 = wstage.tile([128, KC, CQK], F32)
            nc.sync.dma_start(out=wq_f32, in_=Wq.rearrange("(o p) d -> p o d", p=128))
            wk_f32 = wstage.tile([128, KC, CQK], F32)
            nc.sync.dma_start(out=wk_f32, in_=Wk.rearrange("(o p) d -> p o d", p=128))
            wv_f32 = wstage.tile([128, KC, C], F32)
            nc.sync.dma_start(out=wv_f32, in_=Wv.rearrange("(o p) d -> p o d", p=128))
            bv_f32 = wstage.tile([1, C], F32)
            nc.sync.dma_start(out=bv_f32, in_=bv[None, :])
            nc.vector.tensor_copy(out=wqk_sb[:, :, :CQK], in_=wq_f32)
            nc.vector.tensor_copy(out=wqk_sb[:, :, CQK:], in_=wk_f32)
            nc.vector.tensor_copy(out=wv_sb, in_=wv_f32)
            nc.vector.tensor_copy(out=bv_row, in_=bv_f32)

        bqk_sb = singles.tile([128, 1], F32)
        nc.sync.dma_start(out=bqk_sb[:CQK, :], in_=bq[:, None])
        nc.sync.dma_start(out=bqk_sb[CQK:, :], in_=bk[:, None])
        gamma_sb = singles.tile([128, 1], F32)
        nc.sync.dma_start(
            out=gamma_sb,
            in_=bass.AP(
                tensor=gamma.tensor, offset=gamma.offset, ap=[[0, 128], [1, 1]]
            ),
        )

        # ---- persistent per-image state ----
        qkT_sb = singles.tile([128, NPIX], FP16)  # q rows 0:64, k rows 64:128
        qT_sb = qkT_sb[:CQK, :]
        kT_sb = singles.tile([CQK, NPIX], FP16)  # k copied down to partitions 0:64
        # att_v normalized by Zv (values O(1) -> fp8-safe), [w, h, c]
        yvT_img = singles.tile([96, H, C], F8)
        e2_all = singles.tile([96, W, 96], BF16)  # exp scores [g, w, h], diag=0
        zv_img = singles.tile([96, W], F32)  # [h, w]
        rzv_img = singles.tile([96, W], F32)  # 1/Zv, [h, w]
        zv_T = singles.tile([96, 96], BF16)  # [w, h]
        zh_all = singles.tile([96, H], F32)  # [u, r]
        rzg_all = singles.tile([96, H], F32)  # gamma/(Zv+Zh), [u, r]
        v_hbm = dram.tile([NPIX, C], BF16)

        qT_v = qT_sb.rearrange("d (h w) -> d h w", w=W)
        kT_v = kT_sb.rearrange("d (h w) -> d h w", w=W)
        v_img_view = v_hbm.rearrange("(h w) c -> h w c", w=W)

        # ================= Phase 1: projections =================
        x_r = x.rearrange("(t s p) c -> t p s c", s=PS, p=128)
        vout_r = v_hbm.rearrange("(t s p) c -> t p s c", s=PS, p=128)
        with (
            tc.tile_pool(name="p1", bufs=2) as p1,
            tc.tile_pool(name="psA", bufs=4, space="PSUM") as psA,
            tc.tile_pool(name="psB", bufs=3, space="PSUM") as psB,
        ):
            xT_ts = {}

            def p1_stage_a(t):
                # load + transpose tile t into xT (fp16 for q/k, fp8 for v)
                x_t = p1.tile([128, PS, C], FP16, name="x_t", bufs=3)
                nc.gpsimd.dma_start(out=x_t, in_=x_r[t])
                xT_t = p1.tile([128, KC, PT], FP16, name="xT_t", bufs=3)
                for s in range(PS):
                    tp_ps = psA.tile(
                        [128, KC, 128], FP16, name="tp_ps", tag="tp", bufs=2
                    )
                    for cc in range(KC):
                        nc.tensor.transpose(
                            tp_ps[:, cc, :], x_t[:, s, ts(cc, 128)], identity_h
                        )
                    if s % 2 == 0:
                        nc.scalar.copy(out=xT_t[:, :, ts(s, 128)], in_=tp_ps)
                    else:
                        nc.vector.tensor_copy(out=xT_t[:, :, ts(s, 128)], in_=tp_ps)
                xT_ts[t] = xT_t

            def p1_stage_b(t):
                xT_t = xT_ts.pop(t)
                qkp = psB.tile([128, PT], F32, name="qkp", tag="qk", bufs=3)
                for cc in range(KC):
                    nc.tensor.matmul(
                        qkp,
                        lhsT=wqk_sb[:, cc, :],
                        rhs=xT_t[:, cc, :],
                        start=(cc == 0),
                        stop=(cc == KC - 1),
                    )
                nc.scalar.activation(
                    out=qkT_sb[:, ts(t, PT)],
                    in_=qkp,
                    func=mybir.ActivationFunctionType.Identity,
                    bias=bqk_sb,
                    scale=1.0,
                )
                # v projection (fp16) + K=1 bias matmul: [128 pix, 512]
                v_st = p1.tile([128, PS, C], BF16, name="v_st")
                for m in range(PS):
                    vp = psA.tile([128, C], F32, name="vp", tag="mm", bufs=3)
                    for cc in range(KC):
                        nc.tensor.matmul(
                            vp,
                            lhsT=xT_t[:, cc, ts(m, 128)],
                            rhs=wv_sb[:, cc, :],
                            start=(cc == 0),
                            stop=False,
                        )
                    nc.tensor.matmul(
                        vp, lhsT=ones1, rhs=bv_row, start=False, stop=True
                    )
                    if m % 3 == 2:
                        nc.scalar.copy(out=v_st[:, m, :], in_=vp)
                    else:
                        nc.vector.tensor_copy(out=v_st[:, m, :], in_=vp)
                nc.sync.dma_start(out=vout_r[t], in_=v_st)

            p1_stage_a(0)
            for t in range(NT):
                if t + 1 < NT:
                    p1_stage_a(t + 1)
                p1_stage_b(t)
            # k half of qkT -> partitions 0:64 (one SBUF shift DMA)
            nc.sync.dma_start(out=kT_sb, in_=qkT_sb[CQK:, :])

        # ===== Stages A/B/P3 share one pool set so they pipeline =====
        x_rows = x.rearrange("(rb r u) c -> rb u r c", r=RB, u=W)
        y_rows = y.rearrange("(rb r u) c -> rb u r c", r=RB, u=W)
        v_rows = v_hbm.rearrange("(rb r u) c -> rb u r c", r=RB, u=W)
        NB3 = H // RB
        with (
            tc.tile_pool(name="pp", bufs=3) as pp,
            tc.tile_pool(name="pf", bufs=8) as pf,
            tc.tile_pool(name="psS", bufs=2, space="PSUM") as psS,
        ):
            # ---- Stage A: all column scores -> E2_all, Zv, 1/Zv ----
            for wg in range(W // 4):
                w = wg * 4
                sp = psS.tile([96, 4, 96], F32, name="sp", tag="sp", bufs=3)
                for j in range(4):
                    nc.tensor.matmul(
                        sp[:, j, :], lhsT=kT_v[:, :, w + j], rhs=qT_v[:, :, w + j]
                    )
                nc.scalar.activation(
                    out=e2_all[:, w : w + 4, :],
                    in_=sp,
                    func=mybir.ActivationFunctionType.Exp,
                )
                nc.vector.tensor_mul(
                    out=e2_all[:, w : w + 4, :],
                    in0=e2_all[:, w : w + 4, :],
                    in1=antidiag[:, None, :].to_broadcast((96, 4, 96)),
                )
                zp = psS.tile([96, 4], F32, name="zp", tag="zp", bufs=2)
                for j in range(4):
                    nc.tensor.matmul(
                        zp[:, j : j + 1], lhsT=e2_all[:, w + j, :], rhs=ones96
                    )
                nc.vector.tensor_copy(out=zv_img[:, w : w + 4], in_=zp)
                nc.vector.reciprocal(
                    out=rzv_img[:, w : w + 4], in_=zv_img[:, w : w + 4]
                )
            # transpose Zv image once: [h, w] -> [w, h]
            ztp = psS.tile([96, 96], F32, name="ztp", tag="sp", bufs=3)
            nc.tensor.transpose(ztp, zv_img, identity128[:96, :96])
            nc.vector.tensor_copy(out=zv_T, in_=ztp)

            # ---- Stage B: yv matmuls, prescale by 1/Zv, fp8 funnel ----
            for wg in range(W // 4):
                w = wg * 4
                vcol = pp.tile([96, 4, C], BF16, name="vcol", bufs=4)
                nc.sync.dma_start(out=vcol, in_=v_img_view[:, ds(w, 4), :])
                yv_stage = pp.tile([96, 4, C], F8, name="yv_stage", bufs=8)
                for j in range(4):
                    yvp = psS.tile([96, C], F32, name="yvp", tag="mm", bufs=3)
                    nc.tensor.matmul(
                        yvp, lhsT=e2_all[:, w + j, :], rhs=vcol[:, j, :]
                    )
                    if j % 2 == 0:
                        nc.vector.tensor_scalar_mul(
                            out=yv_stage[:, j, :],
                            in0=yvp,
                            scalar1=rzv_img[:, ds(w + j, 1)],
                        )
                    else:
                        nc.scalar.activation(
                            out=yv_stage[:, j, :],
                            in_=yvp,
                            func=mybir.ActivationFunctionType.Copy,
                            scale=rzv_img[:, ds(w + j, 1)],
                        )
                for j in range(4):
                    eng = [nc.sync, nc.gpsimd][(wg * 4 + j) % 2]
                    eng.dma_start(
                        out=yvT_img[w + j : w + j + 1, :, :],
                        in_=yv_stage[:, j, :],
                    )

            # ---- P3: rows + combine ----
            e3_blks = {}

            def p3_scores(rb):
                e3_blk = pp.tile([96, RB, 96], BF16, name="e3_blk", bufs=4)
                sp3 = psS.tile([96, RB, 96], F32, name="sp3", tag="sp", bufs=3)
                for ri in range(RB):
                    r = rb * RB + ri
                    nc.tensor.matmul(
                        sp3[:, ri, :], lhsT=kT_v[:, r, :], rhs=qT_v[:, r, :]
                    )
                nc.scalar.activation(
                    out=e3_blk,
                    in_=sp3,
                    func=mybir.ActivationFunctionType.Exp,
                )
                e3_blks[rb] = e3_blk

            def p3_prefetch(rb):
                vrow = pf.tile([96, RB, C], BF16, name="vrow", bufs=4)
                nc.sync.dma_start(out=vrow, in_=v_rows[rb])
                xrow = pf.tile([96, RB, C], FP16, name="xrow", bufs=4)
                nc.gpsimd.dma_start(out=xrow, in_=x_rows[rb])
                return vrow, xrow

            def p3_consume(rb, vrow, xrow):
                e3_blk = e3_blks.pop(rb)
                y_st = pp.tile([96, RB, C], F32, name="y_st", bufs=3)
                # diag(Zv) tiles for un-scaling yvT rows (built on gpsimd)
                diag4 = pp.tile([96, RB, 96], BF16, name="diag4", bufs=3)
                nc.gpsimd.affine_select(
                    out=diag4,
                    in_=zv_T[:, ts(rb, RB), None].to_broadcast((96, RB, 96)),
                    compare_op=mybir.AluOpType.is_equal,
                    fill=0.0,
                    base=0,
                    pattern=[[0, RB], [-1, 96]],
                    channel_multiplier=1,
                )
                # Z for the whole block, then batched gamma/(Zv+Zh)
                zp3 = psS.tile([96, RB], F32, name="zp3", tag="zp", bufs=2)
                for ri in range(RB):
                    nc.tensor.matmul(
                        zp3[:, ri : ri + 1], lhsT=e3_blk[:, ri, :], rhs=ones96
                    )
                nc.vector.tensor_copy(out=zh_all[:, ts(rb, RB)], in_=zp3)
                rzg_blk = rzg_all[:, ts(rb, RB)]
                nc.vector.tensor_add(
                    out=rzg_blk,
                    in0=zh_all[:, ts(rb, RB)],
                    in1=zv_T[:, ts(rb, RB)],
                )
                nc.vector.reciprocal(out=rzg_blk, in_=rzg_blk)
                nc.vector.tensor_scalar_mul(
                    out=rzg_blk, in0=rzg_blk, scalar1=gamma_sb[:96, :]
                )
                for ri in range(RB):
                    r = rb * RB + ri
                    yp = psS.tile([96, C], F32, name="yp", tag="mm", bufs=3)
                    nc.tensor.matmul(yp, lhsT=e3_blk[:, ri, :], rhs=vrow[:, ri, :],
                                     start=True, stop=False)
                    nc.tensor.matmul(yp, lhsT=diag4[:, ri, :], rhs=yvT_img[:, r, :],
                                     start=False, stop=True)
                    nc.vector.scalar_tensor_tensor(
                        out=y_st[:, ri, :],
                        in0=yp,
                        scalar=rzg_all[:, ds(r, 1)],
                        in1=xrow[:, ri, :],
                        op0=mult,
                        op1=add,
                    )
                nc.sync.dma_start(out=y_rows[rb], in_=y_st)

            p3_scores(0)
            p3_scores(1)
            pfs = {rb: p3_prefetch(rb) for rb in range(min(4, NB3))}
            for rb in range(NB3):
                if rb + 2 < NB3:
                    p3_scores(rb + 2)
                if rb + 4 < NB3:
                    pfs[rb + 4] = p3_prefetch(rb + 4)
                p3_consume(rb, *pfs.pop(rb))



_NC_CACHE = None


def _get_nc():
    global _NC_CACHE
    if _NC_CACHE is None:
        _NC_CACHE = build_nc()
    return _NC_CACHE


def run(inputs: dict, trace: bool = False):
    """Run on 8 cores; returns (full_output [8,96,96,512] f32, BassKernelResults)."""
    x = np.ascontiguousarray(np.asarray(inputs["x"], dtype=np.float32))
    B = x.shape[0]
    assert x.shape == (N_CORES, H, W, C), x.shape
    common = {
        "Wq": np.ascontiguousarray(np.asarray(inputs["Wq"], np.float32)),
        "bq": np.ascontiguousarray(np.asarray(inputs["bq"], np.float32)),
        "Wk": np.ascontiguousarray(np.asarray(inputs["Wk"], np.float32)),
        "bk": np.ascontiguousarray(np.asarray(inputs["bk"], np.float32)),
        "Wv": np.ascontiguousarray(np.asarray(inputs["Wv"], np.float32)),
        "bv": np.ascontiguousarray(np.asarray(inputs["bv"], np.float32)),
        "gamma": np.asarray(inputs["gamma"], np.float32).reshape(1, 1).copy(),
    }
    in_maps = [
        {"x": x[b].reshape(NPIX, C), **common} for b in range(B)
    ]
    nc = _get_nc()
    res = run_bass_kernel_spmd(
        nc, in_maps, core_ids=list(range(N_CORES)), trace=trace
    )
    out = np.stack(
        [res.results[b]["y"].reshape(H, W, C) for b in range(B)], axis=0
    )
    return out, res


def kernel(**inputs) -> np.ndarray:
    out, _ = run(inputs, trace=False)
    return out


if __name__ == "__main__":
    nc = build_nc()
    print("built ok")

